# Initial kernel scaffold
#
"""Trainium2 Bass kernel for nn_EquivariantAttentionLayer.

Reference computation (N=128 frames, P=256 points, D=128, OUT=256, HEADS=16, HD=16):
  qkv  = einsum('ijd,qdhm->qhmij', x, W1)         # temporal QKV
  s1   = einsum('abij,abIj->aiIj', q, k); a1 = softmax(s1, axis=I)
  t    = einsum('aiIj,abIj->abij', a1, v)
  qkv2 = einsum('hmij,qhmgn->qgnij', t, W2)       # point QKV (mix over both head axes)
  s2   = einsum('abij,abiJ->aijJ', q2, k2); a2 = softmax(s2, axis=J)
  pa   = einsum('aijJ,abiJ->ijab', a2, v2).reshape(N,P,256)
  out  = (pa @ fc1_w + fc1_b) @ fc2_w + fc2_b     # NO nonlinearity -> collapses to one 256x256 matmul

Sharding: phase A is point-sharded (temporal attention is independent per point),
phase B/C are frame-sharded (point attention is independent per frame). Two
half-sized AllToAlls re-shard t from point-shards to frame-shards, overlapped
with compute. The FC pair is collapsed on the host (no activation between):
  Wc = fc1_w @ fc2_w ; bc = fc1_b @ fc2_w + fc2_b.
Points are processed in a permuted order (j' = hc*128 + s*16 + jc16); the host
un-permutes the output rows. Heads are processed in PERM order; the host
permutes W2/Wc rows to match.
"""

import numpy as np

# ---- problem dims (hardcoded) ----
NF, NP, D = 128, 256, 128       # frames (i/I), points (j/J), input dim
A_, B_ = 16, 16                 # HD (a/g), HEADS (b/n)
F = A_ * B_                     # 256 features
NCORE = 8
PC = NP // NCORE                # 32 points per core (phase A)
HC = PC // 2                    # 16 points per exchange half
NI = NF // NCORE                # 16 frames per core (phase B)
TOK = NF * PC                   # 4096 tokens per core (both phases)

# Head-processing order: batch bh handles PE row groups {2bh, 2bh+1} so that
# same-PSUM-bank score matmuls are always same-group (HW: cross-group same-bank
# PE writes are fatal).
PERM = [4 * (k // 2) + 2 * bh + (k % 2) for bh in range(2) for k in range(8)]

# Point order as seen by phase B / the raw device output (host un-permutes).
JPERM = np.array([s * PC + hc * HC + jc
                  for hc in range(2) for s in range(NCORE) for jc in range(HC)])


def build_program(phases="AB", n_cores=NCORE, reps=1):
    """Build the SPMD Bass program. phases in {"AB", "A", "B"} (A/B for testing).
    reps>1 repeats the whole body (for wall-clock delta timing)."""
    import concourse.bacc as bacc
    import concourse.mybir as mybir
    import concourse.tile as tile
    from concourse.masks import make_identity

    dt = mybir.dt
    f32 = dt.float32
    f32r = dt.float32r
    f16 = dt.float16

    nc = bacc.Bacc(None, target_bir_lowering=False, num_devices=n_cores)

    if "A" in phases:
        x_d = nc.dram_tensor("x", [NF, PC, D], f32, kind="ExternalInput")
        w1qk_d = nc.dram_tensor("w1qk", [D, 8 * 128], f16, kind="ExternalInput")
        w1v_d = nc.dram_tensor("w1v", [D, F], f16, kind="ExternalInput")
    if "B" in phases:
        w2qk_d = nc.dram_tensor("w2qk", [F, 8 * 128], f16, kind="ExternalInput")
        w2v_d = nc.dram_tensor("w2v", [F, F], f16, kind="ExternalInput")
        wc_d = nc.dram_tensor("wc", [F, F], f32r, kind="ExternalInput")
        bc_d = nc.dram_tensor("bc", [128, F], f32, kind="ExternalInput")
        out_d = nc.dram_tensor("out", [NI, NP, F], f32, kind="ExternalOutput")

    # exchange buffers (per half): tsh[s, f, il, jc16] = t[f, i=s*NI+il, jc]
    kindA = "ExternalOutput" if phases == "A" else None
    kindB = "ExternalInput" if phases == "B" else None
    tsh_ds = tex_ds = None
    if "A" in phases:
        tsh_ds = [[nc.dram_tensor(f"tsh{r}_{h}", [NCORE, F, NI, HC], f16,
                                  **({"kind": kindA} if kindA else {}))
                   for h in range(2)] for r in range(reps)]
    if phases == "AB":
        tex_ds = [[nc.dram_tensor(f"tex{r}_{h}", [NCORE, F, NI, HC], f16)
                   for h in range(2)] for r in range(reps)]
    elif phases == "B":
        tex_ds = [[nc.dram_tensor(f"tex0_{h}", [NCORE, F, NI, HC], f16,
                                  kind="ExternalInput") for h in range(2)]]

    with tile.TileContext(nc) as tc:
        with tc.tile_pool(name="consts", bufs=1) as consts:
            ident = consts.tile([128, 128], f32, tag="ident")
            make_identity(nc, ident[:])

            for r in range(reps):
                if "A" in phases:
                    def do_coll(h, _r=r):
                        if phases != "AB":
                            return
                        nc.gpsimd.collective_compute(
                            "AllToAll", mybir.AluOpType.bypass,
                            replica_groups=[list(range(n_cores))],
                            ins=[tsh_ds[_r][h][:]], outs=[tex_ds[_r][h][:]])
                    _phase_a(nc, tc, tsh_ds[r], do_coll, x_d, w1qk_d, w1v_d,
                             ident, mybir)
                if "B" in phases:
                    _phase_b(nc, tc, out_d, tex_ds[min(r, len(tex_ds) - 1)],
                             w2qk_d, w2v_d, wc_d, bc_d, ident, mybir)

    nc.compile()
    return nc


def _phase_a(nc, tc, tsh_d, do_coll, x_d, w1qk_d, w1v_d, ident, mybir):
    """Temporal QKV + temporal attention for this core's PC points."""
    dt = mybir.dt
    f32, f16, bf16, f32r = dt.float32, dt.float16, dt.bfloat16, dt.float32r
    Exp = mybir.ActivationFunctionType.Exp
    Copy = mybir.ActivationFunctionType.Copy
    MUL = mybir.AluOpType.mult

    with tc.tile_pool(name="a_sb", bufs=1) as sb, \
         tc.tile_pool(name="a_ld", bufs=1) as ld, \
         tc.tile_pool(name="a_exp", bufs=3) as expp, \
         tc.tile_pool(name="a_psm", bufs=2, space="PSUM") as psm, \
         tc.tile_pool(name="a_pss", bufs=2, space="PSUM") as pss, \
         tc.tile_pool(name="a_psv", bufs=2, space="PSUM") as psv:

        def evict(i, out_ap, in_ap):
            # PSUM evictions: 2/3 DVE, 1/3 ACT (ACT carries the softmax exps)
            if i % 3 != 2:
                nc.vector.tensor_copy(out_ap, in_ap)
            else:
                nc.scalar.activation(out_ap, in_ap, Copy)

        w1qk_sb = sb.tile([128, 8 * 128], f16, tag="w1qk")
        nc.sync.dma_start(w1qk_sb[:], w1qk_d[:])
        w1v_sb = sb.tile([128, F], f16, tag="w1v")
        nc.sync.dma_start(w1v_sb[:], w1v_d[:])

        # xt_all[d, j*128+i] = x[i, j, d]
        xt_all = sb.tile([128, TOK], f16, tag="xt")
        xl_all = ld.tile([128, TOK], f32, tag="xl", bufs=1)  # [i, (j, d)]
        xl_r = xl_all[:].rearrange("p (j d) -> p j d", j=PC)
        for q in range(4):
            nc.sync.dma_start(xl_r[:, q * 8:(q + 1) * 8, :],
                              x_d[:, q * 8:(q + 1) * 8, :])
        for j in range(PC):
            ps = psm.tile([128, 512], f32, tag="m")
            nc.tensor.transpose(ps[:, 0:128], xl_r[:, j, :], ident[:])
            evict(j, xt_all[:, j * 128:(j + 1) * 128], ps[:, 0:128])

        # Q/K, padded head layout: tile c (of 8) holds a in [4c,4c+4), partition
        # 32*(a%4)+b (rows +16..31 zero), free = (j, i). c 0-3 = q, 4-7 = k.
        qk = [sb.tile([128, TOK], f16, tag=f"qk{c}", name=f"qk{c}") for c in range(8)]
        # V^T (+ones col): vt[i, (j, a, 17)]; col 16 of each (j,a) block is 1.0
        vt = sb.tile([128, PC * A_ * 17], bf16, tag="vt")
        nc.gpsimd.memset(vt[:], 1.0)
        vt_r = vt[:].rearrange("p (j a c) -> p j a c", j=PC, a=A_, c=17)
        ei = 0
        for nt in range(TOK // 512):
            for c in range(8):
                ps = psm.tile([128, 512], f32, tag="m")
                nc.tensor.matmul(ps[:],
                                 w1qk_sb[:, c * 128:(c + 1) * 128],
                                 xt_all[:, nt * 512:(nt + 1) * 512],
                                 start=True, stop=True)
                evict(ei, qk[c][:, nt * 512:(nt + 1) * 512], ps[:])
                ei += 1
            for j in range(nt * 4, (nt + 1) * 4):
                ps = psv.tile([128, 512], f32, tag="v")
                nc.tensor.matmul(ps[:, 0:F],
                                 xt_all[:, j * 128:(j + 1) * 128],
                                 w1v_sb[:], start=True, stop=True)
                evict(ei, vt_r[:, j, :, 0:16],
                      ps[:, 0:F].rearrange("p (a b) -> p a b", a=A_))
                ei += 1

        # temporal attention; tu[i, (j, pos, b)] unnormalized (pos = PERM'd head
        # order), rz[i, (j, pos)] = 1/Z.
        tu = sb.tile([128, PC * F], f32, tag="tu")
        rz = sb.tile([128, PC * A_], f32, tag="rz")
        # tab[h]: feature-major t; free = (jh, i, jc16)
        tab = [sb.tile([128, TOK], f16, tag=f"tab{h}", name=f"tab{h}")
               for h in range(2)]
        colof = lambda k: (k % 2) * 512 + (k // 2) * 128  # bank = row group

        def emit_scores1(j, bh):
            sps = pss.tile([128, 1024], f32, tag="s", name="sps")
            for k in range(8):
                a = PERM[bh * 8 + k]
                c, s4 = a // 4, a % 4
                tp = (96, 0) if s4 == 3 else None
                # S'[I, i]: lhsT=K (b,I), rhs=Q (b,i)
                nc.tensor.matmul(
                    sps[:, colof(k):colof(k) + 128],
                    qk[4 + c][32 * s4:32 * s4 + 16, j * 128:(j + 1) * 128],
                    qk[c][32 * s4:32 * s4 + 16, j * 128:(j + 1) * 128],
                    start=True, stop=True, tile_position=tp)
            return sps

        def emit_av1(j, bh, sps):
            aex = expp.tile([128, 1024], bf16, tag="aex", name="aex")
            nc.scalar.activation(aex[:], sps[:], Exp)
            tps = psv.tile([128, 512], f32, tag="v", name="tps")
            for k in range(8):
                a = PERM[bh * 8 + k]
                # t^T[i, (b,Z)] = A'^T @ [V^T | 1]
                nc.tensor.matmul(tps[:, k * 17:k * 17 + 17],
                                 aex[:, colof(k):colof(k) + 128],
                                 vt_r[:, j, a, :], start=True, stop=True)
            tr = tps[:, 0:136].rearrange("p (s c) -> p s c", s=8, c=17)
            o = j * A_ + bh * 8
            nc.vector.reciprocal(rz[:, o:o + 8], tr[:, :, 16])
            nc.vector.tensor_copy(
                tu[:, j * F + bh * 128: j * F + bh * 128 + 128]
                  .rearrange("p (a b) -> p a b", a=8),
                tr[:, :, 0:16])

        for jh in range(2):
            prev = None
            for jc in range(HC):
                j = jh * HC + jc
                for bh in range(2):
                    sps = emit_scores1(j, bh)
                    if prev is not None:
                        emit_av1(prev[0], prev[1], prev[2])
                    prev = (j, bh, sps)
            emit_av1(prev[0], prev[1], prev[2])

            # normalize this half: t = tu * rz (broadcast over b)
            rz_b = rz[:, jh * HC * A_:(jh + 1) * HC * A_] \
                .rearrange("p (j a) -> p j a ()", j=HC).broadcast_to(
                    [128, HC, A_, B_])
            tu_r = tu[:, jh * HC * F:(jh + 1) * HC * F] \
                .rearrange("p (j a b) -> p j a b", j=HC, a=A_)
            nc.vector.tensor_tensor(tu_r, tu_r, rz_b, op=MUL)

            # transpose this half to feature-major and ship it
            for jc in range(HC):
                j = jh * HC + jc
                for h in range(2):
                    ps = psm.tile([128, 512], f32, tag="m")
                    nc.tensor.transpose(ps[:, 0:128],
                                        tu[:, j * F + h * 128: j * F + (h + 1) * 128],
                                        ident[:])
                    out_ap = tab[h][:, jh * 2048:(jh + 1) * 2048] \
                        .rearrange("p (i j) -> p i j", j=HC)[:, :, jc]
                    nc.vector.tensor_copy(out_ap, ps[:, 0:128])
            for h in range(2):
                nc.sync.dma_start(
                    tsh_d[jh][:, h * 128:(h + 1) * 128, :, :]
                        .rearrange("s f i j -> f s (i j)"),
                    tab[h][:, jh * 2048:(jh + 1) * 2048]
                        .rearrange("p (s ij) -> p s ij", s=NCORE))
            do_coll(jh)


def _phase_b(nc, tc, out_d, tex_d, w2qk_d, w2v_d, wc_d, bc_d, ident, mybir):
    """Point mix + point attention + collapsed FC for this core's NI frames.

    Token order is (il, j') with j' = hc*128 + s*16 + jc16 (host un-permutes)."""
    dt = mybir.dt
    f32, f16, bf16, f32r = dt.float32, dt.float16, dt.bfloat16, dt.float32r
    Exp = mybir.ActivationFunctionType.Exp
    Copy = mybir.ActivationFunctionType.Copy
    MUL = mybir.AluOpType.mult
    ADD = mybir.AluOpType.add

    with tc.tile_pool(name="b_sb", bufs=1) as sb, \
         tc.tile_pool(name="b_exp", bufs=3) as expp, \
         tc.tile_pool(name="b_out", bufs=3) as outp, \
         tc.tile_pool(name="b_psm", bufs=2, space="PSUM") as psm, \
         tc.tile_pool(name="b_pss", bufs=2, space="PSUM") as pss, \
         tc.tile_pool(name="b_psv", bufs=2, space="PSUM") as psv:

        def evict(i, out_ap, in_ap):
            if i % 3 != 2:
                nc.vector.tensor_copy(out_ap, in_ap)
            else:
                nc.scalar.activation(out_ap, in_ap, Copy)

        w2v_sb = sb.tile([128, 2 * F], f16, tag="w2v")  # col block kt = rows kt*128..
        nc.sync.dma_start(w2v_sb[:, 0:F], w2v_d[0:128, :])
        nc.sync.dma_start(w2v_sb[:, F:2 * F], w2v_d[128:256, :])
        wc_sb = sb.tile([128, 2 * F], f32r, tag="wc")
        nc.sync.dma_start(wc_sb[:, 0:F], wc_d[0:128, :])
        nc.sync.dma_start(wc_sb[:, F:2 * F], wc_d[128:256, :])
        bias_sb = sb.tile([128, F], f32, tag="bias")
        nc.sync.dma_start(bias_sb[:], bc_d[:])

        # q2/k2 padded head layout; free = (hc, il, s, jc16) = (hc, il, j'128)
        q2k2 = [sb.tile([128, TOK], f16, tag=f"q2k2_{c}", name=f"q2k2_{c}")
                for c in range(8)]
        # v2t[hc][j'_loc, (il, a, 17)]
        v2t = [sb.tile([128, NI * A_ * 17], bf16, tag=f"v2t{h}", name=f"v2t{h}")
               for h in range(2)]
        for h in range(2):
            nc.gpsimd.memset(v2t[h][:], 1.0)
        v2t_r = [v2t[h][:].rearrange("p (i a c) -> p i a c", i=NI, a=A_)
                 for h in range(2)]
        q2k2_r = [q2k2[c][:].rearrange("p (hc il j) -> p hc il j", hc=2, il=NI)
                  for c in range(8)]

        with tc.tile_pool(name="b_t2", bufs=1) as t2p:
            w2qk_sb = [t2p.tile([128, 1024], f16, tag=f"w2qk{kt}",
                                name=f"w2qk{kt}") for kt in range(2)]
            for kt in range(2):
                nc.sync.dma_start(w2qk_sb[kt][:],
                                  w2qk_d[kt * 128:(kt + 1) * 128, :])
            # t2[h][f_local, (hc, il, s, jc16)]
            t2 = [t2p.tile([128, TOK], f16, tag=f"t2_{h}", name=f"t2_{h}")
                  for h in range(2)]
            for hc in range(2):
                for h in range(2):
                    t2_v = t2[h][:, hc * 2048:(hc + 1) * 2048] \
                        .rearrange("p (il s j) -> p il s j", il=NI, s=NCORE)
                    for s in range(NCORE):
                        nc.sync.dma_start(
                            t2_v[:, :, s, :],
                            tex_d[hc][s, h * 128:(h + 1) * 128, :, :])
                # mixes for this half
                for nt in range(4):  # il-quads within the half
                    for c in range(8):
                        ps = psm.tile([128, 512], f32, tag="m")
                        for kt in range(2):
                            nc.tensor.matmul(
                                ps[:],
                                w2qk_sb[kt][:, c * 128:(c + 1) * 128],
                                t2[kt][:, hc * 2048 + nt * 512:
                                       hc * 2048 + (nt + 1) * 512],
                                start=(kt == 0), stop=(kt == 1))
                        evict(nt, q2k2_r[c][:, hc, nt * 4:(nt + 1) * 4, :]
                              .rearrange("p il j -> p (il j)"), ps[:])
                for il in range(NI):
                    ps = psv.tile([128, 512], f32, tag="v")
                    for kt in range(2):
                        nc.tensor.matmul(
                            ps[:, 0:F],
                            t2[kt][:, hc * 2048 + il * 128:
                                   hc * 2048 + (il + 1) * 128],
                            w2v_sb[:, kt * F:(kt + 1) * F],
                            start=(kt == 0), stop=(kt == 1))
                    evict(il, v2t_r[hc][:, il, :, 0:16],
                          ps[:, 0:F].rearrange("p (a b) -> p a b", a=A_))

        # point attention: pa_tok[jh][j'_loc, (il, pos, n)] unnorm; rz2 = 1/Z
        pa_tok = [sb.tile([128, NI * F], f32, tag=f"pat{jh}", name=f"pat{jh}")
                  for jh in range(2)]
        rz2 = [sb.tile([128, NI * A_], f32, tag=f"rz2_{jh}", name=f"rz2_{jh}")
               for jh in range(2)]
        def emit_scores2(il, bh, m):
            sps = pss.tile([128, 1024], f32, tag="s", name="sps2")
            for kp in range(2):
                k = m * 2 + kp
                a = PERM[bh * 8 + k]
                c, s4 = a // 4, a % 4
                tp = (96, 0) if s4 == 3 else None
                for Jh in range(2):
                    # lhsT=K2 (n, J'_chunk), rhs=Q2 (n, j'=256)
                    nc.tensor.matmul(
                        sps[:, kp * 512 + Jh * 256: kp * 512 + Jh * 256 + 256],
                        q2k2_r[4 + c][32 * s4:32 * s4 + 16, Jh, il, :],
                        q2k2_r[c][32 * s4:32 * s4 + 16, :, il, :],
                        start=True, stop=True, tile_position=tp)
            return sps

        def emit_av2(il, bh, m, sps, tps):
            aex = expp.tile([128, 1024], bf16, tag="aex2", name="aex2")
            nc.scalar.activation(aex[:], sps[:], Exp)
            for kp in range(2):
                k = m * 2 + kp
                a = PERM[bh * 8 + k]
                for jh in range(2):
                    for Jh in range(2):
                        # pa^T[j'_chunk, (n,Z)] = A2'^T @ [V2^T | 1]
                        nc.tensor.matmul(
                            tps[:, (k * 2 + jh) * 17: (k * 2 + jh) * 17 + 17],
                            aex[:, kp * 512 + Jh * 256 + jh * 128:
                                kp * 512 + Jh * 256 + jh * 128 + 128],
                            v2t_r[Jh][:, il, a, :],
                            start=(Jh == 0), stop=(Jh == 1))

        def drain2(il, bh, tps):
            tr = tps[:, 0:272].rearrange("p (s c) -> p s c", s=16, c=17)
            for jh in range(2):
                o = il * A_ + bh * 8
                nc.vector.reciprocal(rz2[jh][:, o:o + 8], tr[:, jh::2, 16])
                nc.vector.tensor_copy(
                    pa_tok[jh][:, il * F + bh * 128: il * F + bh * 128 + 128]
                        .rearrange("p (a b) -> p a b", a=8),
                    tr[:, jh::2, 0:16])

        # per-il tail: normalize + transpose to feature-major + FC + store
        def il_tail(il):
            pa_f = [sb.tile([128, NP], f32r, tag=f"paf{ah}", name=f"paf{ah}",
                            bufs=2) for ah in range(2)]
            for jh in range(2):
                rz_b = rz2[jh][:, il * A_:(il + 1) * A_] \
                    .rearrange("p a -> p a ()").broadcast_to([128, A_, B_])
                pa_r = pa_tok[jh][:, il * F:(il + 1) * F] \
                    .rearrange("p (a b) -> p a b", a=A_)
                nc.vector.tensor_tensor(pa_r, pa_r, rz_b, op=MUL)
            for jh in range(2):
                for ah in range(2):
                    ps = psm.tile([128, 512], f32, tag="m")
                    nc.tensor.transpose(
                        ps[:, 0:128],
                        pa_tok[jh][:, il * F + ah * 128: il * F + (ah + 1) * 128],
                        ident[:])
                    nc.vector.tensor_copy(
                        pa_f[ah][:, jh * 128:(jh + 1) * 128], ps[:, 0:128])
            for jh in range(2):
                ps = psm.tile([128, 512], f32, tag="m")
                for kt in range(2):
                    nc.tensor.matmul(
                        ps[:, 0:F],
                        pa_f[kt][:, jh * 128:(jh + 1) * 128],
                        wc_sb[:, kt * F:(kt + 1) * F],
                        start=(kt == 0), stop=(kt == 1))
                ot = outp.tile([128, F], f32, tag="ot")
                nc.vector.tensor_tensor(ot[:], ps[:, 0:F], bias_sb[:], op=ADD)
                nc.sync.dma_start(out_d[il, jh * 128:(jh + 1) * 128, :], ot[:])

        prev = None
        tps_map = {}
        for il in range(NI):
            for bh in range(2):
                tps = psv.tile([128, 512], f32, tag="v", name="tps2")
                tps_map[(il, bh)] = tps
                for m in range(4):
                    sps = emit_scores2(il, bh, m)
                    if prev is not None:
                        emit_av2(prev[0], prev[1], prev[2], prev[3],
                                 tps_map[(prev[0], prev[1])])
                        if prev[2] == 3:
                            drain2(prev[0], prev[1], tps_map.pop((prev[0], prev[1])))
                            if prev[1] == 1:
                                il_tail(prev[0])
                    prev = (il, bh, m, sps)
        emit_av2(prev[0], prev[1], prev[2], prev[3], tps_map[(prev[0], prev[1])])
        drain2(prev[0], prev[1], tps_map.pop((prev[0], prev[1])))
        il_tail(prev[0])



# ---------------------------------------------------------------------------
# host side
# ---------------------------------------------------------------------------

def _pad_heads(w, n_in):
    """(n_in, F) with cols f=(a,b) -> (n_in, 4*128): chunk c holds a in
    [4c,4c+4) at col 32*(a%4)+b, cols +16..31 zero."""
    out = np.zeros((n_in, 4 * 128), dtype=np.float32)
    w = w.reshape(n_in, A_, B_)
    for a in range(A_):
        c, s4 = a // 4, a % 4
        out[:, c * 128 + 32 * s4: c * 128 + 32 * s4 + B_] = w[:, a, :]
    return out


def prep_inputs(x, W1, W2, fc1_w, fc1_b, fc2_w, fc2_b):
    """Host-side weight prep + per-core input maps."""
    x = np.asarray(x, dtype=np.float32)
    W1 = np.asarray(W1, dtype=np.float32)
    W2 = np.asarray(W2, dtype=np.float32)
    fc1_w = np.asarray(fc1_w, dtype=np.float32)
    fc1_b = np.asarray(fc1_b, dtype=np.float32)
    fc2_w = np.asarray(fc2_w, dtype=np.float32)
    fc2_b = np.asarray(fc2_b, dtype=np.float32)

    w1q = _pad_heads(W1[0].reshape(D, F), D)
    w1k = _pad_heads(W1[1].reshape(D, F), D)
    w1qk = np.concatenate([w1q, w1k], axis=1).astype(np.float16)
    w1v = W1[2].reshape(D, F).astype(np.float16)

    row_perm = np.array([PERM[pos] * B_ + b for pos in range(A_)
                         for b in range(B_)])
    w2q = _pad_heads(W2[0].reshape(F, F)[row_perm], F)
    w2k = _pad_heads(W2[1].reshape(F, F)[row_perm], F)
    w2qk = np.concatenate([w2q, w2k], axis=1).astype(np.float16)
    w2v = np.ascontiguousarray(W2[2].reshape(F, F)[row_perm]).astype(np.float16)

    wc = np.ascontiguousarray((fc1_w @ fc2_w)[row_perm]).astype(np.float32)
    bc = (fc1_b @ fc2_w + fc2_b).astype(np.float32)
    bc_rep = np.ascontiguousarray(np.broadcast_to(bc, (128, F)))

    in_maps = []
    for s in range(NCORE):
        in_maps.append({
            "x": np.ascontiguousarray(x[:, s * PC:(s + 1) * PC, :]),
            "w1qk": w1qk, "w1v": w1v,
            "w2qk": w2qk, "w2v": w2v,
            "wc": wc, "bc": bc_rep,
        })
    return in_maps


_CACHE = {}


def kernel(**inputs):
    from concourse.bass_utils import run_bass_kernel_spmd

    in_maps = prep_inputs(**inputs)
    if "nc" not in _CACHE:
        _CACHE["nc"] = build_program("AB", NCORE)
    nc = _CACHE["nc"]
    res = run_bass_kernel_spmd(nc, in_maps, list(range(NCORE)))
    out = np.empty((NF, NP, F), dtype=np.float32)
    for s in range(NCORE):
        out[s * NI:(s + 1) * NI, JPERM, :] = res.results[s]["out"]
    return out



# revision 1
# speedup vs baseline: 6.8328x; 6.8328x over previous
"""Trainium2 Bass kernel for nn_EquivariantAttentionLayer.

Reference computation (N=128 frames, P=256 points, D=128, OUT=256, HEADS=16, HD=16):
  qkv  = einsum('ijd,qdhm->qhmij', x, W1)         # temporal QKV
  s1   = einsum('abij,abIj->aiIj', q, k); a1 = softmax(s1, axis=I)
  t    = einsum('aiIj,abIj->abij', a1, v)
  qkv2 = einsum('hmij,qhmgn->qgnij', t, W2)       # point QKV (mix over both head axes)
  s2   = einsum('abij,abiJ->aijJ', q2, k2); a2 = softmax(s2, axis=J)
  pa   = einsum('aijJ,abiJ->ijab', a2, v2).reshape(N,P,256)
  out  = (pa @ fc1_w + fc1_b) @ fc2_w + fc2_b     # NO nonlinearity -> collapses to one 256x256 matmul

Sharding: phase A is point-sharded (temporal attention is independent per point),
phase B/C are frame-sharded (point attention is independent per frame). Two
half-sized AllToAlls re-shard t from point-shards to frame-shards, overlapped
with compute. The FC pair is collapsed on the host (no activation between):
  Wc = fc1_w @ fc2_w ; bc = fc1_b @ fc2_w + fc2_b.
Points are processed in a permuted order (j' = hc*128 + s*16 + jc16); the host
un-permutes the output rows. Heads are processed in PERM order; the host
permutes W2/Wc rows to match.
"""

import numpy as np

# ---- problem dims (hardcoded) ----
NF, NP, D = 128, 256, 128       # frames (i/I), points (j/J), input dim
A_, B_ = 16, 16                 # HD (a/g), HEADS (b/n)
F = A_ * B_                     # 256 features
NCORE = 8
PC = NP // NCORE                # 32 points per core (phase A)
HC = PC // 2                    # 16 points per exchange half
NI = NF // NCORE                # 16 frames per core (phase B)
TOK = NF * PC                   # 4096 tokens per core (both phases)

# Head-processing order: batch bh handles PE row groups {2bh, 2bh+1} so that
# same-PSUM-bank score matmuls are always same-group (HW: cross-group same-bank
# PE writes are fatal).
PERM = [4 * (k // 2) + 2 * bh + (k % 2) for bh in range(2) for k in range(8)]

# Point order as seen by phase B / the raw device output (host un-permutes).
JPERM = np.array([s * PC + hc * HC + jc
                  for hc in range(2) for s in range(NCORE) for jc in range(HC)])


def build_program(phases="AB", n_cores=NCORE, reps=1):
    """Build the SPMD Bass program. phases in {"AB", "A", "B"} (A/B for testing).
    reps>1 repeats the whole body (for wall-clock delta timing)."""
    import concourse.bacc as bacc
    import concourse.mybir as mybir
    import concourse.tile as tile
    from concourse.masks import make_identity

    dt = mybir.dt
    f32 = dt.float32
    f32r = dt.float32r
    f16 = dt.float16

    nc = bacc.Bacc(None, target_bir_lowering=False, num_devices=n_cores)

    if "A" in phases:
        x_d = nc.dram_tensor("x", [NF, PC, D], f32, kind="ExternalInput")
        w1qk_d = nc.dram_tensor("w1qk", [D, 8 * 128], f16, kind="ExternalInput")
        w1v_d = nc.dram_tensor("w1v", [D, F], f16, kind="ExternalInput")
    if "B" in phases:
        w2qk_d = nc.dram_tensor("w2qk", [F, 8 * 128], f16, kind="ExternalInput")
        w2v_d = nc.dram_tensor("w2v", [F, F], f16, kind="ExternalInput")
        wc_d = nc.dram_tensor("wc", [F, F], f32r, kind="ExternalInput")
        bc_d = nc.dram_tensor("bc", [128, F], f32, kind="ExternalInput")
        out_d = nc.dram_tensor("out", [NI, NP, F], f32, kind="ExternalOutput")

    # exchange buffers (per half): tsh[s, f, il, jc16] = t[f, i=s*NI+il, jc]
    kindA = "ExternalOutput" if phases == "A" else None
    kindB = "ExternalInput" if phases == "B" else None
    tsh_ds = tex_ds = None
    if "A" in phases:
        tsh_ds = [[nc.dram_tensor(f"tsh{r}_{h}", [NCORE, F, NI, HC], f16,
                                  **({"kind": kindA} if kindA else {}))
                   for h in range(2)] for r in range(reps)]
    if phases == "AB":
        tex_ds = [[nc.dram_tensor(f"tex{r}_{h}", [NCORE, F, NI, HC], f16)
                   for h in range(2)] for r in range(reps)]
    elif phases == "B":
        tex_ds = [[nc.dram_tensor(f"tex0_{h}", [NCORE, F, NI, HC], f16,
                                  kind="ExternalInput") for h in range(2)]]

    with tile.TileContext(nc) as tc:
        with tc.tile_pool(name="consts", bufs=1) as consts:
            ident = consts.tile([128, 128], f32, tag="ident")
            make_identity(nc, ident[:])

            for r in range(reps):
                if "A" in phases:
                    def do_coll(h, _r=r):
                        if phases != "AB":
                            return
                        nc.gpsimd.collective_compute(
                            "AllToAll", mybir.AluOpType.bypass,
                            replica_groups=[list(range(n_cores))],
                            ins=[tsh_ds[_r][h][:]], outs=[tex_ds[_r][h][:]])
                    _phase_a(nc, tc, tsh_ds[r], do_coll, x_d, w1qk_d, w1v_d,
                             ident, mybir)
                if "B" in phases:
                    _phase_b(nc, tc, out_d, tex_ds[min(r, len(tex_ds) - 1)],
                             w2qk_d, w2v_d, wc_d, bc_d, ident, mybir)

    nc.compile()
    return nc


def _phase_a(nc, tc, tsh_d, do_coll, x_d, w1qk_d, w1v_d, ident, mybir):
    """Temporal QKV + temporal attention for this core's PC points."""
    dt = mybir.dt
    f32, f16, bf16, f32r = dt.float32, dt.float16, dt.bfloat16, dt.float32r
    Exp = mybir.ActivationFunctionType.Exp
    Copy = mybir.ActivationFunctionType.Copy
    MUL = mybir.AluOpType.mult

    with tc.tile_pool(name="a_sb", bufs=1) as sb, \
         tc.tile_pool(name="a_ld", bufs=1) as ld, \
         tc.tile_pool(name="a_exp", bufs=3) as expp, \
         tc.tile_pool(name="a_psm", bufs=2, space="PSUM") as psm, \
         tc.tile_pool(name="a_pss", bufs=2, space="PSUM") as pss, \
         tc.tile_pool(name="a_psv", bufs=2, space="PSUM") as psv:

        def evict(i, out_ap, in_ap):
            # PSUM evictions: 2/3 DVE, 1/3 ACT (ACT carries the softmax exps)
            if i % 3 != 2:
                nc.vector.tensor_copy(out_ap, in_ap)
            else:
                nc.scalar.activation(out_ap, in_ap, Copy)

        w1qk_sb = sb.tile([128, 8 * 128], f16, tag="w1qk")
        nc.sync.dma_start(w1qk_sb[:], w1qk_d[:])
        w1v_sb = sb.tile([128, F], f16, tag="w1v")
        nc.sync.dma_start(w1v_sb[:], w1v_d[:])

        # xt_all[d, j*128+i] = x[i, j, d]
        xt_all = sb.tile([128, TOK], f16, tag="xt")
        xl_all = ld.tile([128, TOK], f32, tag="xl", bufs=1)  # [i, (j, d)]
        xl_r = xl_all[:].rearrange("p (j d) -> p j d", j=PC)
        for q in range(4):
            nc.sync.dma_start(xl_r[:, q * 8:(q + 1) * 8, :],
                              x_d[:, q * 8:(q + 1) * 8, :])
        for j in range(PC):
            ps = psm.tile([128, 512], f32, tag="m")
            nc.tensor.transpose(ps[:, 0:128], xl_r[:, j, :], ident[:])
            evict(j, xt_all[:, j * 128:(j + 1) * 128], ps[:, 0:128])

        # Q/K, padded head layout: tile c (of 8) holds a in [4c,4c+4), partition
        # 32*(a%4)+b (rows +16..31 zero), free = (j, i). c 0-3 = q, 4-7 = k.
        qk = [sb.tile([128, TOK], f16, tag=f"qk{c}", name=f"qk{c}") for c in range(8)]
        # V^T (+ones col): vt[i, (j, a, 17)]; col 16 of each (j,a) block is 1.0
        vt = sb.tile([128, PC * A_ * 17], bf16, tag="vt")
        nc.gpsimd.memset(vt[:], 1.0)
        vt_r = vt[:].rearrange("p (j a c) -> p j a c", j=PC, a=A_, c=17)
        ei = 0
        for nt in range(TOK // 512):
            for c in range(8):
                ps = psm.tile([128, 512], f32, tag="m")
                nc.tensor.matmul(ps[:],
                                 w1qk_sb[:, c * 128:(c + 1) * 128],
                                 xt_all[:, nt * 512:(nt + 1) * 512],
                                 start=True, stop=True)
                evict(ei, qk[c][:, nt * 512:(nt + 1) * 512], ps[:])
                ei += 1
            for j in range(nt * 4, (nt + 1) * 4):
                ps = psv.tile([128, 512], f32, tag="v")
                nc.tensor.matmul(ps[:, 0:F],
                                 xt_all[:, j * 128:(j + 1) * 128],
                                 w1v_sb[:], start=True, stop=True)
                evict(ei, vt_r[:, j, :, 0:16],
                      ps[:, 0:F].rearrange("p (a b) -> p a b", a=A_))
                ei += 1

        # temporal attention; tu[i, (j, pos, b)] unnormalized (pos = PERM'd head
        # order), rz[i, (j, pos)] = 1/Z.
        tu = sb.tile([128, PC * F], f32, tag="tu")
        rz = sb.tile([128, PC * A_], f32, tag="rz")
        # tab[h]: feature-major t; free = (jh, i, jc16)
        tab = [sb.tile([128, TOK], f16, tag=f"tab{h}", name=f"tab{h}")
               for h in range(2)]
        colof = lambda k: (k % 2) * 512 + (k // 2) * 128  # bank = row group

        def emit_scores1(j, bh):
            sps = pss.tile([128, 1024], f32, tag="s", name="sps")
            for k in range(8):
                a = PERM[bh * 8 + k]
                c, s4 = a // 4, a % 4
                tp = (96, 0) if s4 == 3 else None
                # S'[I, i]: lhsT=K (b,I), rhs=Q (b,i)
                nc.tensor.matmul(
                    sps[:, colof(k):colof(k) + 128],
                    qk[4 + c][32 * s4:32 * s4 + 16, j * 128:(j + 1) * 128],
                    qk[c][32 * s4:32 * s4 + 16, j * 128:(j + 1) * 128],
                    start=True, stop=True, tile_position=tp)
            return sps

        def emit_av1(j, bh, sps):
            aex = expp.tile([128, 1024], bf16, tag="aex", name="aex")
            nc.scalar.activation(aex[:], sps[:], Exp)
            tps = psv.tile([128, 512], f32, tag="v", name="tps")
            for k in range(8):
                a = PERM[bh * 8 + k]
                # t^T[i, (b,Z)] = A'^T @ [V^T | 1]
                nc.tensor.matmul(tps[:, k * 17:k * 17 + 17],
                                 aex[:, colof(k):colof(k) + 128],
                                 vt_r[:, j, a, :], start=True, stop=True)
            tr = tps[:, 0:136].rearrange("p (s c) -> p s c", s=8, c=17)
            o = j * A_ + bh * 8
            nc.vector.reciprocal(rz[:, o:o + 8], tr[:, :, 16])
            nc.vector.tensor_copy(
                tu[:, j * F + bh * 128: j * F + bh * 128 + 128]
                  .rearrange("p (a b) -> p a b", a=8),
                tr[:, :, 0:16])

        for jh in range(2):
            prev = None
            for jc in range(HC):
                j = jh * HC + jc
                for bh in range(2):
                    sps = emit_scores1(j, bh)
                    if prev is not None:
                        emit_av1(prev[0], prev[1], prev[2])
                    prev = (j, bh, sps)
            emit_av1(prev[0], prev[1], prev[2])

            # normalize this half: t = tu * rz (broadcast over b)
            rz_b = rz[:, jh * HC * A_:(jh + 1) * HC * A_] \
                .rearrange("p (j a) -> p j a ()", j=HC).broadcast_to(
                    [128, HC, A_, B_])
            tu_r = tu[:, jh * HC * F:(jh + 1) * HC * F] \
                .rearrange("p (j a b) -> p j a b", j=HC, a=A_)
            nc.vector.tensor_tensor(tu_r, tu_r, rz_b, op=MUL)

            # transpose this half to feature-major and ship it
            for jc in range(HC):
                j = jh * HC + jc
                for h in range(2):
                    ps = psm.tile([128, 512], f32, tag="m")
                    nc.tensor.transpose(ps[:, 0:128],
                                        tu[:, j * F + h * 128: j * F + (h + 1) * 128],
                                        ident[:])
                    out_ap = tab[h][:, jh * 2048:(jh + 1) * 2048] \
                        .rearrange("p (i j) -> p i j", j=HC)[:, :, jc]
                    nc.vector.tensor_copy(out_ap, ps[:, 0:128])
            for h in range(2):
                nc.sync.dma_start(
                    tsh_d[jh][:, h * 128:(h + 1) * 128, :, :]
                        .rearrange("s f i j -> f s (i j)"),
                    tab[h][:, jh * 2048:(jh + 1) * 2048]
                        .rearrange("p (s ij) -> p s ij", s=NCORE))
            do_coll(jh)


def _phase_b(nc, tc, out_d, tex_d, w2qk_d, w2v_d, wc_d, bc_d, ident, mybir):
    """Point mix + point attention + collapsed FC for this core's NI frames.

    Token order is (il, j') with j' = hc*128 + s*16 + jc16 (host un-permutes)."""
    dt = mybir.dt
    f32, f16, bf16, f32r = dt.float32, dt.float16, dt.bfloat16, dt.float32r
    Exp = mybir.ActivationFunctionType.Exp
    Copy = mybir.ActivationFunctionType.Copy
    MUL = mybir.AluOpType.mult
    ADD = mybir.AluOpType.add

    with tc.tile_pool(name="b_sb", bufs=1) as sb, \
         tc.tile_pool(name="b_exp", bufs=3) as expp, \
         tc.tile_pool(name="b_out", bufs=3) as outp, \
         tc.tile_pool(name="b_psm", bufs=2, space="PSUM") as psm, \
         tc.tile_pool(name="b_pss", bufs=2, space="PSUM") as pss, \
         tc.tile_pool(name="b_psv", bufs=2, space="PSUM") as psv:

        def evict(i, out_ap, in_ap):
            if i % 3 != 2:
                nc.vector.tensor_copy(out_ap, in_ap)
            else:
                nc.scalar.activation(out_ap, in_ap, Copy)

        w2v_sb = sb.tile([128, 2 * F], f16, tag="w2v")  # col block kt = rows kt*128..
        nc.sync.dma_start(w2v_sb[:, 0:F], w2v_d[0:128, :])
        nc.sync.dma_start(w2v_sb[:, F:2 * F], w2v_d[128:256, :])
        wc_sb = sb.tile([128, 2 * F], f32r, tag="wc")
        nc.sync.dma_start(wc_sb[:, 0:F], wc_d[0:128, :])
        nc.sync.dma_start(wc_sb[:, F:2 * F], wc_d[128:256, :])
        bias_sb = sb.tile([128, F], f32, tag="bias")
        nc.sync.dma_start(bias_sb[:], bc_d[:])

        # q2/k2 padded head layout; free = (hc, il, s, jc16) = (hc, il, j'128)
        q2k2 = [sb.tile([128, TOK], f16, tag=f"q2k2_{c}", name=f"q2k2_{c}")
                for c in range(8)]
        # v2t[hc][j'_loc, (il, a, 17)]
        v2t = [sb.tile([128, NI * A_ * 17], bf16, tag=f"v2t{h}", name=f"v2t{h}")
               for h in range(2)]
        for h in range(2):
            nc.gpsimd.memset(v2t[h][:], 1.0)
        v2t_r = [v2t[h][:].rearrange("p (i a c) -> p i a c", i=NI, a=A_)
                 for h in range(2)]
        q2k2_r = [q2k2[c][:].rearrange("p (hc il j) -> p hc il j", hc=2, il=NI)
                  for c in range(8)]

        with tc.tile_pool(name="b_t2", bufs=1) as t2p:
            w2qk_sb = [t2p.tile([128, 1024], f16, tag=f"w2qk{kt}",
                                name=f"w2qk{kt}") for kt in range(2)]
            for kt in range(2):
                nc.sync.dma_start(w2qk_sb[kt][:],
                                  w2qk_d[kt * 128:(kt + 1) * 128, :])
            # t2[h][f_local, (hc, il, s, jc16)]
            t2 = [t2p.tile([128, TOK], f16, tag=f"t2_{h}", name=f"t2_{h}")
                  for h in range(2)]
            for hc in range(2):
                for h in range(2):
                    t2_v = t2[h][:, hc * 2048:(hc + 1) * 2048] \
                        .rearrange("p (il s j) -> p il s j", il=NI, s=NCORE)
                    for s in range(NCORE):
                        nc.sync.dma_start(
                            t2_v[:, :, s, :],
                            tex_d[hc][s, h * 128:(h + 1) * 128, :, :])
                # mixes for this half
                for nt in range(4):  # il-quads within the half
                    for c in range(8):
                        ps = psm.tile([128, 512], f32, tag="m")
                        for kt in range(2):
                            nc.tensor.matmul(
                                ps[:],
                                w2qk_sb[kt][:, c * 128:(c + 1) * 128],
                                t2[kt][:, hc * 2048 + nt * 512:
                                       hc * 2048 + (nt + 1) * 512],
                                start=(kt == 0), stop=(kt == 1))
                        evict(nt, q2k2_r[c][:, hc, nt * 4:(nt + 1) * 4, :]
                              .rearrange("p il j -> p (il j)"), ps[:])
                for il in range(NI):
                    ps = psv.tile([128, 512], f32, tag="v")
                    for kt in range(2):
                        nc.tensor.matmul(
                            ps[:, 0:F],
                            t2[kt][:, hc * 2048 + il * 128:
                                   hc * 2048 + (il + 1) * 128],
                            w2v_sb[:, kt * F:(kt + 1) * F],
                            start=(kt == 0), stop=(kt == 1))
                    evict(il, v2t_r[hc][:, il, :, 0:16],
                          ps[:, 0:F].rearrange("p (a b) -> p a b", a=A_))

        # point attention: pa_tok[jh][j'_loc, (il, pos, n)] unnorm; rz2 = 1/Z
        pa_tok = [sb.tile([128, NI * F], f32, tag=f"pat{jh}", name=f"pat{jh}")
                  for jh in range(2)]
        rz2 = [sb.tile([128, NI * A_], f32, tag=f"rz2_{jh}", name=f"rz2_{jh}")
               for jh in range(2)]
        def emit_scores2(il, bh, m):
            sps = pss.tile([128, 1024], f32, tag="s", name="sps2")
            for kp in range(2):
                k = m * 2 + kp
                a = PERM[bh * 8 + k]
                c, s4 = a // 4, a % 4
                tp = (96, 0) if s4 == 3 else None
                for Jh in range(2):
                    # lhsT=K2 (n, J'_chunk), rhs=Q2 (n, j'=256)
                    nc.tensor.matmul(
                        sps[:, kp * 512 + Jh * 256: kp * 512 + Jh * 256 + 256],
                        q2k2_r[4 + c][32 * s4:32 * s4 + 16, Jh, il, :],
                        q2k2_r[c][32 * s4:32 * s4 + 16, :, il, :],
                        start=True, stop=True, tile_position=tp)
            return sps

        def emit_av2(il, bh, m, sps, tps):
            aex = expp.tile([128, 1024], bf16, tag="aex2", name="aex2")
            nc.scalar.activation(aex[:], sps[:], Exp)
            for kp in range(2):
                k = m * 2 + kp
                a = PERM[bh * 8 + k]
                for jh in range(2):
                    for Jh in range(2):
                        # pa^T[j'_chunk, (n,Z)] = A2'^T @ [V2^T | 1]
                        nc.tensor.matmul(
                            tps[:, (k * 2 + jh) * 17: (k * 2 + jh) * 17 + 17],
                            aex[:, kp * 512 + Jh * 256 + jh * 128:
                                kp * 512 + Jh * 256 + jh * 128 + 128],
                            v2t_r[Jh][:, il, a, :],
                            start=(Jh == 0), stop=(Jh == 1))

        def drain2(il, bh, tps):
            tr = tps[:, 0:272].rearrange("p (s c) -> p s c", s=16, c=17)
            for jh in range(2):
                o = il * A_ + bh * 8
                nc.vector.reciprocal(rz2[jh][:, o:o + 8], tr[:, jh::2, 16])
                nc.vector.tensor_copy(
                    pa_tok[jh][:, il * F + bh * 128: il * F + bh * 128 + 128]
                        .rearrange("p (a b) -> p a b", a=8),
                    tr[:, jh::2, 0:16])

        # per-il tail: normalize + transpose to feature-major + FC + store
        def il_tail(il):
            pa_f = [sb.tile([128, NP], f32r, tag=f"paf{ah}", name=f"paf{ah}",
                            bufs=2) for ah in range(2)]
            for jh in range(2):
                rz_b = rz2[jh][:, il * A_:(il + 1) * A_] \
                    .rearrange("p a -> p a ()").broadcast_to([128, A_, B_])
                pa_r = pa_tok[jh][:, il * F:(il + 1) * F] \
                    .rearrange("p (a b) -> p a b", a=A_)
                nc.vector.tensor_tensor(pa_r, pa_r, rz_b, op=MUL)
            for jh in range(2):
                for ah in range(2):
                    ps = psm.tile([128, 512], f32, tag="m")
                    nc.tensor.transpose(
                        ps[:, 0:128],
                        pa_tok[jh][:, il * F + ah * 128: il * F + (ah + 1) * 128],
                        ident[:])
                    nc.vector.tensor_copy(
                        pa_f[ah][:, jh * 128:(jh + 1) * 128], ps[:, 0:128])
            for jh in range(2):
                ps = psm.tile([128, 512], f32, tag="m")
                for kt in range(2):
                    nc.tensor.matmul(
                        ps[:, 0:F],
                        pa_f[kt][:, jh * 128:(jh + 1) * 128],
                        wc_sb[:, kt * F:(kt + 1) * F],
                        start=(kt == 0), stop=(kt == 1))
                ot = outp.tile([128, F], f32, tag="ot")
                nc.vector.tensor_tensor(ot[:], ps[:, 0:F], bias_sb[:], op=ADD)
                nc.sync.dma_start(out_d[il, jh * 128:(jh + 1) * 128, :], ot[:])

        prev = None
        tps_map = {}
        for il in range(NI):
            for bh in range(2):
                tps = psv.tile([128, 512], f32, tag="v", name="tps2")
                tps_map[(il, bh)] = tps
                for m in range(4):
                    sps = emit_scores2(il, bh, m)
                    if prev is not None:
                        emit_av2(prev[0], prev[1], prev[2], prev[3],
                                 tps_map[(prev[0], prev[1])])
                        if prev[2] == 3:
                            drain2(prev[0], prev[1], tps_map.pop((prev[0], prev[1])))
                            if prev[1] == 1:
                                il_tail(prev[0])
                    prev = (il, bh, m, sps)
        emit_av2(prev[0], prev[1], prev[2], prev[3], tps_map[(prev[0], prev[1])])
        drain2(prev[0], prev[1], tps_map.pop((prev[0], prev[1])))
        il_tail(prev[0])



# ---------------------------------------------------------------------------
# host side
# ---------------------------------------------------------------------------

def _pad_heads(w, n_in):
    """(n_in, F) with cols f=(a,b) -> (n_in, 4*128): chunk c holds a in
    [4c,4c+4) at col 32*(a%4)+b, cols +16..31 zero."""
    out = np.zeros((n_in, 4 * 128), dtype=np.float32)
    w = w.reshape(n_in, A_, B_)
    for a in range(A_):
        c, s4 = a // 4, a % 4
        out[:, c * 128 + 32 * s4: c * 128 + 32 * s4 + B_] = w[:, a, :]
    return out


def prep_inputs(x, W1, W2, fc1_w, fc1_b, fc2_w, fc2_b):
    """Host-side weight prep + per-core input maps."""
    x = np.asarray(x, dtype=np.float32)
    W1 = np.asarray(W1, dtype=np.float32)
    W2 = np.asarray(W2, dtype=np.float32)
    fc1_w = np.asarray(fc1_w, dtype=np.float32)
    fc1_b = np.asarray(fc1_b, dtype=np.float32)
    fc2_w = np.asarray(fc2_w, dtype=np.float32)
    fc2_b = np.asarray(fc2_b, dtype=np.float32)

    w1q = _pad_heads(W1[0].reshape(D, F), D)
    w1k = _pad_heads(W1[1].reshape(D, F), D)
    w1qk = np.concatenate([w1q, w1k], axis=1).astype(np.float16)
    w1v = W1[2].reshape(D, F).astype(np.float16)

    row_perm = np.array([PERM[pos] * B_ + b for pos in range(A_)
                         for b in range(B_)])
    w2q = _pad_heads(W2[0].reshape(F, F)[row_perm], F)
    w2k = _pad_heads(W2[1].reshape(F, F)[row_perm], F)
    w2qk = np.concatenate([w2q, w2k], axis=1).astype(np.float16)
    w2v = np.ascontiguousarray(W2[2].reshape(F, F)[row_perm]).astype(np.float16)

    wc = np.ascontiguousarray((fc1_w @ fc2_w)[row_perm]).astype(np.float32)
    bc = (fc1_b @ fc2_w + fc2_b).astype(np.float32)
    bc_rep = np.ascontiguousarray(np.broadcast_to(bc, (128, F)))

    in_maps = []
    for s in range(NCORE):
        in_maps.append({
            "x": np.ascontiguousarray(x[:, s * PC:(s + 1) * PC, :]),
            "w1qk": w1qk, "w1v": w1v,
            "w2qk": w2qk, "w2v": w2v,
            "wc": wc, "bc": bc_rep,
        })
    return in_maps


_CACHE = {}


def kernel(**inputs):
    from concourse.bass_utils import run_bass_kernel_spmd

    in_maps = prep_inputs(**inputs)
    if "nc" not in _CACHE:
        _CACHE["nc"] = build_program("AB", NCORE)
    nc = _CACHE["nc"]
    res = run_bass_kernel_spmd(nc, in_maps, list(range(NCORE)))
    out = np.empty((NF, NP, F), dtype=np.float32)
    for s in range(NCORE):
        out[s * NI:(s + 1) * NI, JPERM, :] = res.results[s]["out"]
    return out

